# revision 18
# baseline (speedup 1.0000x reference)
"""Multi-head attention (B=8, S=1024, E=768, H=12, D=64) on 8 TRN2 NeuronCores.

Sharding: data-parallel over batch. Core i computes batch element i end to end;
weights are replicated. No collectives.

All matmul operands are bf16 (contraction-128 bf16 matmuls stream noticeably
faster than f32r on TRN2; dense PE work also holds the clock at boost).
Weights are DMA'd as f32 into staging tiles and cast to bf16, overlapped with
the x-transpose prelude and the attention loop (cast thunks ride the PE filler
schedule; DVE-only during attention so the ACT exp chain never blocks on
weight DMA).

Attention runs as a flat software pipeline over 96 global chunks (slot s =
(pair, q-half), kc = 128-key block; chunk g = 8s+kc).  The producer emits
scores (PE) + exp (ACT) for chunk g while the consumer accumulates PV for
chunk g-LEAD, so ACT banks LEAD chunks of exp work in an SBUF ring during the
PE-heavy projection phase and the PE never waits on exp in the filler-free
tail.  The last LEAD steps are consume-only (pure PE work).

Normalization uses a ones-block: each head's v_pad slab is [128 keys, 64 ones
cols | 64 v dims], so the PV matmul emits the softmax denominator already
broadcast across psum rows 0-63 (partition base 0, where the custom-DVE
reciprocal requires its input).  Normalize = fast reciprocal + two multiplies
on DVE; no PE broadcast, no denominator row copies.

PSUM budget (8 banks): scores 2x2 + pv 2 (single [128,1024] slot) + mm 2x1.
"""

import numpy as np

import concourse.bass as bass
import concourse.bacc as bacc
import concourse.tile as tile
from concourse import mybir
from concourse.bass_utils import run_bass_kernel_spmd
from concourse.bass_interp import get_hw_module
from concourse.masks import make_identity

F32 = mybir.dt.float32
BF16 = mybir.dt.bfloat16
U32 = mybir.dt.uint32

B, S, E = 8, 1024, 768
H, D = 12, 64
F = 3 * E                  # 2304
NCORES = 8
NPAIR = H // 2             # 6 head pairs
NKC = S // 128             # 8 key chunks
NST = S // 128             # 8 sequence tiles
NE = E // 128              # 6 embedding chunks
NSLOT = 2 * NPAIR          # 12 (pair, q-half) slots
VW = 128                   # per-head v_pad slab: 64 ones cols + 64 v dims
LEAD = 12                  # producer-consumer distance in chunks
NCH = NSLOT * NKC          # 96 chunks

BF16_ONES = 0x3F803F80     # two packed bf16 1.0


def _build():
    nc = bacc.Bacc("TRN2", target_bir_lowering=False, debug=False,
                   num_devices=NCORES)

    x_d = nc.dram_tensor("x", [S, E], F32, kind="ExternalInput").ap()
    wqkv_d = nc.dram_tensor("w_qkv", [E, F], F32, kind="ExternalInput").ap()
    wout_d = nc.dram_tensor("w_out", [E, E], F32, kind="ExternalInput").ap()
    bout_d = nc.dram_tensor("b_out", [E], F32, kind="ExternalInput").ap()
    y_d = nc.dram_tensor("y", [S, E], F32, kind="ExternalOutput").ap()

    with tile.TileContext(nc) as tc:
        _emit(nc, tc, x_d, wqkv_d, wout_d, bout_d, y_d)

    nc.compile()
    nc.m = get_hw_module(nc.m)
    return nc


def _emit(nc, tc, x_d, wqkv_d, wout_d, bout_d, y_d):
    from contextlib import ExitStack
    ctx = ExitStack()
    with ctx:
        singles = ctx.enter_context(tc.tile_pool(name="singles", bufs=1))
        sb = ctx.enter_context(tc.tile_pool(name="sb", bufs=1))
        ps = ctx.enter_context(tc.tile_pool(name="ps", bufs=1, space="PSUM"))
        bcast_pool = ctx.enter_context(tc.tile_pool(name="bcast", bufs=2))
        ypool = ctx.enter_context(tc.tile_pool(name="ypool", bufs=2))
        wpool = ctx.enter_context(tc.tile_pool(name="wpool", bufs=1))

        # ---- constants ----
        identity = singles.tile([128, 128], BF16)
        make_identity(nc, identity)
        bias_bc = singles.tile([128, E], F32)

        wq_pool = tc.alloc_tile_pool(name="wq_pool", bufs=1)
        wst_pool = tc.alloc_tile_pool(name="wst_pool", bufs=1)
        x_pool = tc.alloc_tile_pool(name="x_pool", bufs=1)

        # bf16 weights for QKV projection: wq[ei] holds rows [128*ei, +128)
        wq = [wq_pool.tile([128, F], BF16, name=f"wqkv_{ei}")
              for ei in range(NE)]

        # f32 staging for weight chunks (DMA f32 -> cast bf16).
        def dma_w_group(ei, c0, cn, tag, bufs):
            st_t = wst_pool.tile([128, cn], F32, tag=tag, bufs=bufs,
                                 name=f"wst_{ei}_{c0}")
            nc.sync.dma_start(out=st_t,
                              in_=wqkv_d[ei * 128:(ei + 1) * 128, c0:c0 + cn])
            return st_t

        def cast_w_group(ei, c0, st_t, on_act=False):
            dst = wq[ei][:, c0:c0 + st_t.shape[1]]
            if on_act:
                nc.scalar.copy(dst, st_t)
            else:
                nc.vector.tensor_copy(dst, st_t)

        # v_pad[st]: per head a [128, 128] slab = 64 ones cols | 64 v dims
        # (ones first so the denominator lands at psum partition base 0,
        # where the custom-DVE reciprocal reads it).
        v_pad = [sb.tile([128, H * VW], BF16, name=f"vpad_{st}")
                 for st in range(NST)]
        for st in range(NST):
            nc.gpsimd.memset(v_pad[st].bitcast(U32), BF16_ONES)
        qkT = [sb.tile([128, S], BF16, name=f"qkT_{ft}")
               for ft in range(2 * NE)]

        # ---- x -> bf16 -> PE transpose -> xT [E, S] bf16 ----
        # Half 0 also accumulates the pair-0 qkt chunks (ft 0 and NE) per ei
        # right after each xT drain, so the first scores/exp start early.
        xT = [wq_pool.tile([128, S], BF16, name=f"xT_{ei}")
              for ei in range(NE)]
        w_stage = {}
        ps_qkt = {}
        for half in range(2):
            x_tiles = []
            for k in range(4):
                st = half * 4 + k
                x_t = x_pool.tile([128, E], F32, tag="x", bufs=4,
                                  name=f"x_{st}")
                for q in range(3):
                    nc.sync.dma_start(
                        out=x_t[:, q * 256:(q + 1) * 256],
                        in_=x_d[st * 128:(st + 1) * 128,
                                q * 256:(q + 1) * 256])
                x_tiles.append((st, x_t))
            if half == 0:
                for ei in range(NE):
                    w_stage[(ei, 0)] = dma_w_group(ei, 0, 128, "wsts", 12)
                    w_stage[(ei, E)] = dma_w_group(ei, E, 128, "wsts", 12)
            else:
                for ei in range(NE):
                    w_stage[(ei, 2 * E)] = dma_w_group(ei, 2 * E, E,
                                                       "wstb", 6)
                for ei in range(NE):
                    w_stage[(ei, 128)] = dma_w_group(ei, 128, E - 128,
                                                     "wstb", 6)
                for ei in range(NE):
                    w_stage[(ei, E + 128)] = dma_w_group(ei, E + 128, E - 128,
                                                         "wstb", 6)
            # cast x -> bf16 per 256-col chunk (transposes for ei pair q can
            # start as soon as chunk q of all four tiles landed).  Half-1
            # casts are DVE-only and its xb tiles persist (sb pool): the
            # half-1 transposes run as attention fillers, so ACT reaches the
            # first exp with no half-1 work queued ahead of it.
            xbb = []
            for (st, x_t) in x_tiles:
                if half == 0:
                    xb = x_pool.tile([128, E], BF16, tag="xb", bufs=4,
                                     name=f"xb_{st}")
                else:
                    xb = sb.tile([128, E], BF16, name=f"xb_{st}")
                for q in range(3):
                    src = x_t[:, q * 256:(q + 1) * 256]
                    dst = xb[:, q * 256:(q + 1) * 256]
                    if half == 1 or (st + q) % 2 == 0:
                        nc.vector.tensor_copy(dst, src)
                    else:
                        nc.scalar.copy(dst, src)
                xbb.append(xb)
            if half == 0:
                for ei in range(NE):
                    cast_w_group(ei, 0, w_stage[(ei, 0)],
                                 on_act=(ei % 2 == 0))
                    cast_w_group(ei, E, w_stage[(ei, E)],
                                 on_act=(ei % 2 == 1))
                for ft in (0, NE):
                    ps_qkt[ft] = ps.tile([128, 512], F32, tag="scores",
                                         bufs=2, name=f"psqkt_{ft}")
            if half == 0:
                for ei in range(NE):
                    ps_xt = ps.tile([128, 512], BF16, tag="mm", bufs=2,
                                    name=f"psxt_{ei}_0")
                    for k in range(4):
                        nc.tensor.transpose(
                            ps_xt[:, k * 128:(k + 1) * 128],
                            xbb[k][:, ei * 128:(ei + 1) * 128],
                            identity)
                    dst = xT[ei][:, 0:512]
                    if ei % 2 == 0:
                        nc.vector.tensor_copy(dst, ps_xt)
                    else:
                        nc.scalar.copy(dst, ps_xt)
                    for ft in (0, NE):
                        nc.tensor.matmul(
                            ps_qkt[ft],
                            wq[ei][:, ft * 128:(ft + 1) * 128],
                            xT[ei][:, 0:512],
                            start=(ei == 0), stop=(ei == NE - 1))
                nc.scalar.copy(qkT[0][:, 0:512], ps_qkt[0])
                nc.vector.tensor_copy(qkT[NE][:, 0:512], ps_qkt[NE])
            else:
                xbb_h1 = xbb
        x_pool.release()

        def transp_h1(ei, xbb=None):
            ps_xt = ps.tile([128, 512], BF16, tag="mm", bufs=2,
                            name=f"psxt_{ei}_1")
            for k in range(4):
                nc.tensor.transpose(
                    ps_xt[:, k * 128:(k + 1) * 128],
                    xbb_h1[k][:, ei * 128:(ei + 1) * 128],
                    identity)
            nc.vector.tensor_copy(xT[ei][:, 512:1024], ps_xt)

        # expst ring reuses the released x staging space (opened after
        # x_pool.release(); released before wst/wq below)
        expst_pool = tc.alloc_tile_pool(name="expst_pool", bufs=14)

        # ---- projection chunk emitters (PE fillers) ----
        def emit_v_chunk(st, c0, cn):
            ps_v = ps.tile([128, 512], F32, tag="mm", bufs=2,
                           name=f"psv_{st}_{c0}")
            for ei in range(NE):
                nc.tensor.matmul(
                    ps_v[:, 0:cn],
                    xT[ei][:, st * 128:(st + 1) * 128],
                    wq[ei][:, 2 * E + c0:2 * E + c0 + cn],
                    start=(ei == 0), stop=(ei == NE - 1))
            vp3 = v_pad[st].rearrange("p (h c) -> p h c", c=VW)
            h0 = c0 // D
            nc.vector.tensor_copy(
                vp3[:, h0:h0 + cn // D, D:VW],
                ps_v[:, 0:cn].rearrange("p (h d) -> p h d", d=D))

        def emit_qkt_chunk(ft, sc):
            ps_q = ps.tile([128, 512], F32, tag="mm", bufs=2,
                           name=f"psq_{ft}_{sc}")
            for ei in range(NE):
                nc.tensor.matmul(
                    ps_q,
                    wq[ei][:, ft * 128:(ft + 1) * 128],
                    xT[ei][:, sc * 512:(sc + 1) * 512],
                    start=(ei == 0), stop=(ei == NE - 1))
            nc.vector.tensor_copy(qkT[ft][:, sc * 512:(sc + 1) * 512], ps_q)



        # ---- w_out staging (DMA early, cast via fillers) ----
        wo = [wpool.tile([128, E], BF16, name=f"wout_{ei}")
              for ei in range(NE)]
        wo_stage = {}
        for ei in range(NE):
            st_t = wst_pool.tile([128, E], F32, tag="wstb", bufs=7,
                                 name=f"wost_{ei}")
            nc.sync.dma_start(
                out=st_t, in_=wout_d[ei * 128:(ei + 1) * 128, :])
            wo_stage[ei] = st_t
        nc.sync.dma_start(
            out=bias_bc,
            in_=bass.AP(tensor=bout_d.tensor, offset=bout_d.offset,
                        ap=[[0, 128]] + list(bout_d.ap)))

        # ---- filler schedule: producer step (8*slot + kc) -> thunks ----
        filler_schedule = {}

        def sched(step, thunk):
            filler_schedule.setdefault(step, []).append(thunk)

        def pop_filler(step):
            for thunk in filler_schedule.pop(step, ()):
                thunk()

        # half-1 transposes + V casts as early fillers; weight casts DVE
        for ei in range(NE):
            sched(ei // 2, lambda ei=ei: transp_h1(ei))
        for ei in range(NE):
            sched(ei // 2,
                  lambda ei=ei: cast_w_group(ei, 2 * E, w_stage[(ei, 2 * E)]))
        # upper q/k halves of pair 0 (kT cols 512+ first needed at step 4)
        sched(3, lambda: emit_qkt_chunk(NE, 1))
        sched(3, lambda: emit_qkt_chunk(0, 1))
        for ei in range(NE):
            sched(3 + ei // 3,
                  lambda ei=ei: cast_w_group(ei, 128, w_stage[(ei, 128)]))
        for ei in range(NE):
            sched(5 + ei // 3,
                  lambda ei=ei: cast_w_group(ei, E + 128,
                                             w_stage[(ei, E + 128)]))
        for ei in range(NE):
            sched(33 + 2 * ei,
                  lambda ei=ei: nc.vector.tensor_copy(wo[ei], wo_stage[ei]))
        # V chunks: v_pad[k] consumed by PV chunk k at step k+LEAD
        for st in range(NST):
            step = 2 + st if st < 4 else 4 + st
            sched(step, lambda st=st: emit_v_chunk(st, 0, 512))
            sched(step, lambda st=st: emit_v_chunk(st, 512, 256))
        # qkt chunks for pair j: (j,0) & (NE+j,0) by step 16j; (NE+j,1) by
        # 16j+4; (j,1) by 16j+8
        for j in range(1, NPAIR):
            base = 16 * (j - 1)
            sched(base + 6, lambda j=j: emit_qkt_chunk(j, 0))
            sched(base + 8, lambda j=j: emit_qkt_chunk(NE + j, 0))
            sched(base + 12, lambda j=j: emit_qkt_chunk(NE + j, 1))
            sched(base + 14, lambda j=j: emit_qkt_chunk(j, 1))

        # ---- attention: flat-step pipelined producer/consumer ----
        attnT = [sb.tile([128, S], BF16, name=f"attnT_{j}")
                 for j in range(NPAIR)]
        expst_tiles = {}
        ps_pv_of = {}

        def norm_tail(c):
            j, qh = c // 2, c % 2
            q0 = qh * 512
            ps_pv = ps_pv_of.pop(c)
            bc = bcast_pool.tile([64, 1024], F32, tag="bc", name=f"bc_{c}")
            nc.vector.reciprocal_approx_fast(out=bc, in_=ps_pv[0:64, :])
            for hh in range(2):
                nc.vector.tensor_mul(
                    attnT[j][hh * 64:(hh + 1) * 64, q0:q0 + 512],
                    ps_pv[64:128, hh * 512:(hh + 1) * 512],
                    bc[:, hh * 512:(hh + 1) * 512])

        for g in range(NCH + LEAD):
            if g < NCH:
                s, kc = divmod(g, NKC)
                j, qh = s // 2, s % 2
                q0 = qh * 512
                qT = qkT[j]
                kT = qkT[NE + j]
                expst = expst_pool.tile([128, 1024], BF16, tag="expst",
                                        name=f"expst_{g}")
                expst_tiles[g] = expst
                ps_s = ps.tile([128, 1024], F32, tag="scores", bufs=2,
                               name=f"pss_{g}")
                for hh in range(2):
                    nc.tensor.matmul(
                        ps_s[:, hh * 512:(hh + 1) * 512],
                        kT[hh * 64:(hh + 1) * 64, kc * 128:(kc + 1) * 128],
                        qT[hh * 64:(hh + 1) * 64, q0:q0 + 512],
                        start=True, stop=True,
                        tile_position=(hh * 64, 0))
                nc.scalar.activation(
                    out=expst, in_=ps_s,
                    func=mybir.ActivationFunctionType.Exp,
                    scale=0.125)
            pop_filler(g)
            cg = g - LEAD
            if cg >= 0:
                c, ckc = divmod(cg, NKC)
                cj = c // 2
                if ckc == 0:
                    ps_pv_of[c] = ps.tile([128, 1024], F32, tag="pv",
                                          bufs=1, name=f"pspv_{c}")
                ps_pv = ps_pv_of[c]
                cexp = expst_tiles.pop(cg)
                for hh in range(2):
                    nc.tensor.matmul(
                        ps_pv[:, hh * 512:(hh + 1) * 512],
                        v_pad[ckc][:, (2 * cj + hh) * VW:
                                   (2 * cj + hh + 1) * VW],
                        cexp[:, hh * 512:(hh + 1) * 512],
                        start=(ckc == 0), stop=(ckc == NKC - 1))
                if ckc == NKC - 1:
                    norm_tail(c)
        for key in sorted(filler_schedule):
            for thunk in filler_schedule[key]:
                thunk()
        filler_schedule.clear()
        expst_pool.release()
        wst_pool.release()
        wq_pool.release()

        # ---- output projection + bias ----
        for st in range(NST):
            y_t = ypool.tile([128, E], F32, tag="y", name=f"y_{st}")
            for (c0, cn) in ((0, 512), (512, 256)):
                ps_y = ps.tile([128, 512], F32, tag="mm", bufs=2,
                               name=f"psy_{st}_{c0}")
                for ej in range(NE):
                    nc.tensor.matmul(
                        ps_y[:, 0:cn],
                        attnT[ej][:, st * 128:(st + 1) * 128],
                        wo[ej][:, c0:c0 + cn],
                        start=(ej == 0), stop=(ej == NE - 1))
                nc.vector.tensor_add(y_t[:, c0:c0 + cn], ps_y[:, 0:cn],
                                     bias_bc[:, c0:c0 + cn])
            nc.sync.dma_start(out=y_d[st * 128:(st + 1) * 128, :], in_=y_t)


_NC_CACHE = None


def _get_nc():
    global _NC_CACHE
    if _NC_CACHE is None:
        _NC_CACHE = _build()
    return _NC_CACHE


def kernel(x, w_qkv, w_out, b_out, _trace=False, **_run_kwargs):
    """Full-input MHA: x [8,1024,768] f32 -> y [8,1024,768] f32."""
    nc = _get_nc()
    x = np.ascontiguousarray(np.asarray(x, dtype=np.float32))
    w_qkv = np.ascontiguousarray(np.asarray(w_qkv, dtype=np.float32))
    w_out = np.ascontiguousarray(np.asarray(w_out, dtype=np.float32))
    b_out = np.ascontiguousarray(np.asarray(b_out, dtype=np.float32))
    in_maps = [
        {"x": x[i], "w_qkv": w_qkv, "w_out": w_out, "b_out": b_out}
        for i in range(NCORES)
    ]
    res = run_bass_kernel_spmd(nc, in_maps, core_ids=list(range(NCORES)),
                               trace=_trace, **_run_kwargs)
    y = np.stack([res.results[i]["y"] for i in range(NCORES)], axis=0)
    if _trace:
        return y, res
    return y


# revision 19
# speedup vs baseline: 1.0056x; 1.0056x over previous
"""Multi-head attention (B=8, S=1024, E=768, H=12, D=64) on 8 TRN2 NeuronCores.

Sharding: data-parallel over batch. Core i computes batch element i end to end;
weights are replicated. No collectives.

All matmul operands are bf16 (contraction-128 bf16 matmuls stream noticeably
faster than f32r on TRN2; dense PE work also holds the clock at boost).
Weights are DMA'd as f32 into staging tiles and cast to bf16, overlapped with
the x-transpose prelude and the attention loop (cast thunks ride the PE filler
schedule; DVE-only during attention so the ACT exp chain never blocks on
weight DMA).

Attention runs as a flat software pipeline over 96 global chunks (slot s =
(pair, q-half), kc = 128-key block; chunk g = 8s+kc).  The producer emits
scores (PE) + exp (ACT) for chunk g while the consumer accumulates PV for
chunk g-LEAD, so ACT banks LEAD chunks of exp work in an SBUF ring during the
PE-heavy projection phase and the PE never waits on exp in the filler-free
tail.  The last LEAD steps are consume-only (pure PE work).

Normalization uses a ones-block: each head's v_pad slab is [128 keys, 64 ones
cols | 64 v dims], so the PV matmul emits the softmax denominator already
broadcast across psum rows 0-63 (partition base 0, where the custom-DVE
reciprocal requires its input).  Normalize = fast reciprocal + two multiplies
on DVE; no PE broadcast, no denominator row copies.

PSUM budget (8 banks): scores 2x2 + pv 2 (single [128,1024] slot) + mm 2x1.
"""

import numpy as np

import concourse.bass as bass
import concourse.bacc as bacc
import concourse.tile as tile
from concourse import mybir
from concourse.bass_utils import run_bass_kernel_spmd
from concourse.bass_interp import get_hw_module
from concourse.masks import make_identity

F32 = mybir.dt.float32
BF16 = mybir.dt.bfloat16
U32 = mybir.dt.uint32

B, S, E = 8, 1024, 768
H, D = 12, 64
F = 3 * E                  # 2304
NCORES = 8
NPAIR = H // 2             # 6 head pairs
NKC = S // 128             # 8 key chunks
NST = S // 128             # 8 sequence tiles
NE = E // 128              # 6 embedding chunks
NSLOT = 2 * NPAIR          # 12 (pair, q-half) slots
VW = 128                   # per-head v_pad slab: 64 ones cols + 64 v dims
LEAD = 12                  # producer-consumer distance in chunks
NCH = NSLOT * NKC          # 96 chunks

BF16_ONES = 0x3F803F80     # two packed bf16 1.0


def _build():
    nc = bacc.Bacc("TRN2", target_bir_lowering=False, debug=False,
                   num_devices=NCORES)

    x_d = nc.dram_tensor("x", [S, E], F32, kind="ExternalInput").ap()
    wqkv_d = nc.dram_tensor("w_qkv", [E, F], F32, kind="ExternalInput").ap()
    wout_d = nc.dram_tensor("w_out", [E, E], F32, kind="ExternalInput").ap()
    bout_d = nc.dram_tensor("b_out", [E], F32, kind="ExternalInput").ap()
    y_d = nc.dram_tensor("y", [S, E], F32, kind="ExternalOutput").ap()

    with tile.TileContext(nc) as tc:
        _emit(nc, tc, x_d, wqkv_d, wout_d, bout_d, y_d)

    nc.compile()
    nc.m = get_hw_module(nc.m)
    return nc


def _emit(nc, tc, x_d, wqkv_d, wout_d, bout_d, y_d):
    from contextlib import ExitStack
    ctx = ExitStack()
    with ctx:
        singles = ctx.enter_context(tc.tile_pool(name="singles", bufs=1))
        sb = ctx.enter_context(tc.tile_pool(name="sb", bufs=1))
        ps = ctx.enter_context(tc.tile_pool(name="ps", bufs=1, space="PSUM"))
        bcast_pool = ctx.enter_context(tc.tile_pool(name="bcast", bufs=2))
        ypool = ctx.enter_context(tc.tile_pool(name="ypool", bufs=2))
        wpool = ctx.enter_context(tc.tile_pool(name="wpool", bufs=1))

        # ---- constants ----
        identity = singles.tile([128, 128], BF16)
        make_identity(nc, identity)
        bias_bc = singles.tile([128, E], F32)

        wq_pool = tc.alloc_tile_pool(name="wq_pool", bufs=1)
        wst_pool = tc.alloc_tile_pool(name="wst_pool", bufs=1)
        x_pool = tc.alloc_tile_pool(name="x_pool", bufs=1)

        # bf16 weights for QKV projection: wq[ei] holds rows [128*ei, +128)
        wq = [wq_pool.tile([128, F], BF16, name=f"wqkv_{ei}")
              for ei in range(NE)]

        # f32 staging for weight chunks (DMA f32 -> cast bf16).
        def dma_w_group(ei, c0, cn, tag, bufs):
            st_t = wst_pool.tile([128, cn], F32, tag=tag, bufs=bufs,
                                 name=f"wst_{ei}_{c0}")
            nc.sync.dma_start(out=st_t,
                              in_=wqkv_d[ei * 128:(ei + 1) * 128, c0:c0 + cn])
            return st_t

        def cast_w_group(ei, c0, st_t, on_act=False):
            dst = wq[ei][:, c0:c0 + st_t.shape[1]]
            if on_act:
                nc.scalar.copy(dst, st_t)
            else:
                nc.vector.tensor_copy(dst, st_t)

        # v_pad[st]: per head a [128, 128] slab = 64 ones cols | 64 v dims
        # (ones first so the denominator lands at psum partition base 0,
        # where the custom-DVE reciprocal reads it).
        v_pad = [sb.tile([128, H * VW], BF16, name=f"vpad_{st}")
                 for st in range(NST)]
        for st in range(NST):
            nc.gpsimd.memset(v_pad[st].bitcast(U32), BF16_ONES)
        qkT = [sb.tile([128, S], BF16, name=f"qkT_{ft}")
               for ft in range(2 * NE)]

        # ---- x -> bf16 -> PE transpose -> xT [E, S] bf16 ----
        # Half 0 also accumulates the pair-0 qkt chunks (ft 0 and NE) per ei
        # right after each xT drain, so the first scores/exp start early.
        xT = [wq_pool.tile([128, S], BF16, name=f"xT_{ei}")
              for ei in range(NE)]
        w_stage = {}
        ps_qkt = {}
        for half in range(2):
            x_tiles = []
            for k in range(4):
                st = half * 4 + k
                x_t = x_pool.tile([128, E], F32, tag="x", bufs=4,
                                  name=f"x_{st}")
                for q in range(3):
                    nc.sync.dma_start(
                        out=x_t[:, q * 256:(q + 1) * 256],
                        in_=x_d[st * 128:(st + 1) * 128,
                                q * 256:(q + 1) * 256])
                x_tiles.append((st, x_t))
            if half == 0:
                for ei in range(NE):
                    w_stage[(ei, 0)] = dma_w_group(ei, 0, 128, "wsts", 12)
                    w_stage[(ei, E)] = dma_w_group(ei, E, 128, "wsts", 12)
            else:
                for ei in range(NE):
                    w_stage[(ei, 2 * E)] = dma_w_group(ei, 2 * E, E,
                                                       "wstb", 7)
                for ei in range(NE):
                    w_stage[(ei, 128)] = dma_w_group(ei, 128, E - 128,
                                                     "wstb", 7)
                for ei in range(NE):
                    w_stage[(ei, E + 128)] = dma_w_group(ei, E + 128, E - 128,
                                                         "wstb", 7)
            # cast x -> bf16 per 256-col chunk (transposes for ei pair q can
            # start as soon as chunk q of all four tiles landed)
            xbb = []
            for (st, x_t) in x_tiles:
                xb = x_pool.tile([128, E], BF16, tag="xb", bufs=4,
                                 name=f"xb_{st}")
                for q in range(3):
                    src = x_t[:, q * 256:(q + 1) * 256]
                    dst = xb[:, q * 256:(q + 1) * 256]
                    if (st + q) % 2 == 0:
                        nc.vector.tensor_copy(dst, src)
                    else:
                        nc.scalar.copy(dst, src)
                xbb.append(xb)
            if half == 0:
                for ei in range(NE):
                    cast_w_group(ei, 0, w_stage[(ei, 0)],
                                 on_act=(ei % 2 == 0))
                    cast_w_group(ei, E, w_stage[(ei, E)],
                                 on_act=(ei % 2 == 1))
                for ft in (0, NE):
                    ps_qkt[ft] = ps.tile([128, 512], F32, tag="scores",
                                         bufs=2, name=f"psqkt_{ft}")
            for ei in range(NE):
                ps_xt = ps.tile([128, 512], BF16, tag="mm", bufs=2,
                                name=f"psxt_{ei}_{half}")
                for k in range(4):
                    nc.tensor.transpose(
                        ps_xt[:, k * 128:(k + 1) * 128],
                        xbb[k][:, ei * 128:(ei + 1) * 128],
                        identity)
                dst = xT[ei][:, half * 512:(half + 1) * 512]
                if ei % 2 == 0:
                    nc.vector.tensor_copy(dst, ps_xt)
                else:
                    nc.scalar.copy(dst, ps_xt)
                if half == 0:
                    for ft in (0, NE):
                        nc.tensor.matmul(
                            ps_qkt[ft],
                            wq[ei][:, ft * 128:(ft + 1) * 128],
                            xT[ei][:, 0:512],
                            start=(ei == 0), stop=(ei == NE - 1))
            if half == 0:
                nc.scalar.copy(qkT[0][:, 0:512], ps_qkt[0])
                nc.vector.tensor_copy(qkT[NE][:, 0:512], ps_qkt[NE])
        x_pool.release()

        # expst ring reuses the released x staging space (opened after
        # x_pool.release(); released before wst/wq below)
        expst_pool = tc.alloc_tile_pool(name="expst_pool", bufs=15)

        # ---- projection chunk emitters (PE fillers) ----
        def emit_v_chunk(st, c0, cn):
            ps_v = ps.tile([128, 512], F32, tag="mm", bufs=2,
                           name=f"psv_{st}_{c0}")
            for ei in range(NE):
                nc.tensor.matmul(
                    ps_v[:, 0:cn],
                    xT[ei][:, st * 128:(st + 1) * 128],
                    wq[ei][:, 2 * E + c0:2 * E + c0 + cn],
                    start=(ei == 0), stop=(ei == NE - 1))
            vp3 = v_pad[st].rearrange("p (h c) -> p h c", c=VW)
            h0 = c0 // D
            nc.vector.tensor_copy(
                vp3[:, h0:h0 + cn // D, D:VW],
                ps_v[:, 0:cn].rearrange("p (h d) -> p h d", d=D))

        def emit_qkt_chunk(ft, sc):
            ps_q = ps.tile([128, 512], F32, tag="mm", bufs=2,
                           name=f"psq_{ft}_{sc}")
            for ei in range(NE):
                nc.tensor.matmul(
                    ps_q,
                    wq[ei][:, ft * 128:(ft + 1) * 128],
                    xT[ei][:, sc * 512:(sc + 1) * 512],
                    start=(ei == 0), stop=(ei == NE - 1))
            nc.vector.tensor_copy(qkT[ft][:, sc * 512:(sc + 1) * 512], ps_q)

        # V casts: DVE-only (ACT exp chain must never block on weight DMA)
        for ei in range(NE):
            cast_w_group(ei, 2 * E, w_stage[(ei, 2 * E)], on_act=False)

        # ---- w_out staging (DMA early, cast via fillers) ----
        wo = [wpool.tile([128, E], BF16, name=f"wout_{ei}")
              for ei in range(NE)]
        wo_stage = {}
        for ei in range(NE):
            st_t = wst_pool.tile([128, E], F32, tag="wstb", bufs=7,
                                 name=f"wost_{ei}")
            nc.sync.dma_start(
                out=st_t, in_=wout_d[ei * 128:(ei + 1) * 128, :])
            wo_stage[ei] = st_t
        nc.sync.dma_start(
            out=bias_bc,
            in_=bass.AP(tensor=bout_d.tensor, offset=bout_d.offset,
                        ap=[[0, 128]] + list(bout_d.ap)))

        # ---- filler schedule: producer step (8*slot + kc) -> thunks ----
        filler_schedule = {}

        def sched(step, thunk):
            filler_schedule.setdefault(step, []).append(thunk)

        def pop_filler(step):
            for thunk in filler_schedule.pop(step, ()):
                thunk()

        # weight casts (DVE)
        for ei in range(NE):
            sched(ei // 2,
                  lambda ei=ei: cast_w_group(ei, 128, w_stage[(ei, 128)]))
        for ei in range(NE):
            sched(3 + ei // 2,
                  lambda ei=ei: cast_w_group(ei, E + 128,
                                             w_stage[(ei, E + 128)]))
        for ei in range(NE):
            sched(33 + 2 * ei,
                  lambda ei=ei: nc.vector.tensor_copy(wo[ei], wo_stage[ei]))

        # upper q/k halves of pair 0
        sched(0, lambda: emit_qkt_chunk(0, 1))
        sched(0, lambda: emit_qkt_chunk(NE, 1))
        # V chunks: v_pad[k] consumed by PV chunk k at step k+LEAD
        for st in range(NST):
            step = 1 + st if st < 4 else 4 + st
            sched(step, lambda st=st: emit_v_chunk(st, 0, 512))
            sched(step, lambda st=st: emit_v_chunk(st, 512, 256))
        # qkt chunks for pair j: (j,0) & (NE+j,0) by step 16j; (NE+j,1) by
        # 16j+4; (j,1) by 16j+8
        for j in range(1, NPAIR):
            base = 16 * (j - 1)
            sched(base + 3, lambda j=j: emit_qkt_chunk(j, 0))
            sched(base + 6, lambda j=j: emit_qkt_chunk(NE + j, 0))
            sched(base + 12, lambda j=j: emit_qkt_chunk(NE + j, 1))
            sched(base + 14, lambda j=j: emit_qkt_chunk(j, 1))

        # ---- attention: flat-step pipelined producer/consumer ----
        attnT = [sb.tile([128, S], BF16, name=f"attnT_{j}")
                 for j in range(NPAIR)]
        expst_tiles = {}
        ps_pv_of = {}

        def norm_tail(c):
            j, qh = c // 2, c % 2
            q0 = qh * 512
            ps_pv = ps_pv_of.pop(c)
            bc = bcast_pool.tile([64, 1024], F32, tag="bc", name=f"bc_{c}")
            nc.vector.reciprocal_approx_fast(out=bc, in_=ps_pv[0:64, :])
            for hh in range(2):
                nc.vector.tensor_mul(
                    attnT[j][hh * 64:(hh + 1) * 64, q0:q0 + 512],
                    ps_pv[64:128, hh * 512:(hh + 1) * 512],
                    bc[:, hh * 512:(hh + 1) * 512])

        for g in range(NCH + LEAD):
            if g < NCH:
                s, kc = divmod(g, NKC)
                j, qh = s // 2, s % 2
                q0 = qh * 512
                qT = qkT[j]
                kT = qkT[NE + j]
                expst = expst_pool.tile([128, 1024], BF16, tag="expst",
                                        name=f"expst_{g}")
                expst_tiles[g] = expst
                ps_s = ps.tile([128, 1024], F32, tag="scores", bufs=2,
                               name=f"pss_{g}")
                for hh in range(2):
                    nc.tensor.matmul(
                        ps_s[:, hh * 512:(hh + 1) * 512],
                        kT[hh * 64:(hh + 1) * 64, kc * 128:(kc + 1) * 128],
                        qT[hh * 64:(hh + 1) * 64, q0:q0 + 512],
                        start=True, stop=True,
                        tile_position=(hh * 64, 0))
                nc.scalar.activation(
                    out=expst, in_=ps_s,
                    func=mybir.ActivationFunctionType.Exp,
                    scale=0.125)
            pop_filler(g)
            cg = g - LEAD
            if cg >= 0:
                c, ckc = divmod(cg, NKC)
                cj = c // 2
                if ckc == 0:
                    ps_pv_of[c] = ps.tile([128, 1024], F32, tag="pv",
                                          bufs=1, name=f"pspv_{c}")
                ps_pv = ps_pv_of[c]
                cexp = expst_tiles.pop(cg)
                for hh in range(2):
                    nc.tensor.matmul(
                        ps_pv[:, hh * 512:(hh + 1) * 512],
                        v_pad[ckc][:, (2 * cj + hh) * VW:
                                   (2 * cj + hh + 1) * VW],
                        cexp[:, hh * 512:(hh + 1) * 512],
                        start=(ckc == 0), stop=(ckc == NKC - 1))
                if ckc == NKC - 1:
                    norm_tail(c)
        for key in sorted(filler_schedule):
            for thunk in filler_schedule[key]:
                thunk()
        filler_schedule.clear()
        expst_pool.release()
        wst_pool.release()
        wq_pool.release()

        # ---- output projection + bias ----
        for st in range(NST):
            y_t = ypool.tile([128, E], F32, tag="y", name=f"y_{st}")
            for (c0, cn) in ((0, 512), (512, 256)):
                ps_y = ps.tile([128, 512], F32, tag="mm", bufs=2,
                               name=f"psy_{st}_{c0}")
                for ej in range(NE):
                    nc.tensor.matmul(
                        ps_y[:, 0:cn],
                        attnT[ej][:, st * 128:(st + 1) * 128],
                        wo[ej][:, c0:c0 + cn],
                        start=(ej == 0), stop=(ej == NE - 1))
                nc.vector.tensor_add(y_t[:, c0:c0 + cn], ps_y[:, 0:cn],
                                     bias_bc[:, c0:c0 + cn])
            nc.sync.dma_start(out=y_d[st * 128:(st + 1) * 128, :], in_=y_t)


_NC_CACHE = None


def _get_nc():
    global _NC_CACHE
    if _NC_CACHE is None:
        _NC_CACHE = _build()
    return _NC_CACHE


def kernel(x, w_qkv, w_out, b_out, _trace=False, **_run_kwargs):
    """Full-input MHA: x [8,1024,768] f32 -> y [8,1024,768] f32."""
    nc = _get_nc()
    x = np.ascontiguousarray(np.asarray(x, dtype=np.float32))
    w_qkv = np.ascontiguousarray(np.asarray(w_qkv, dtype=np.float32))
    w_out = np.ascontiguousarray(np.asarray(w_out, dtype=np.float32))
    b_out = np.ascontiguousarray(np.asarray(b_out, dtype=np.float32))
    in_maps = [
        {"x": x[i], "w_qkv": w_qkv, "w_out": w_out, "b_out": b_out}
        for i in range(NCORES)
    ]
    res = run_bass_kernel_spmd(nc, in_maps, core_ids=list(range(NCORES)),
                               trace=_trace, **_run_kwargs)
    y = np.stack([res.results[i]["y"] for i in range(NCORES)], axis=0)
    if _trace:
        return y, res
    return y
